# revision 12
# baseline (speedup 1.0000x reference)
"""CondConv (per-sample dynamic conv) Trainium2 Bass kernel.

Reference computation (per sample b):
    gap     = mean(x[b], spatial)                    # [C]
    r       = sigmoid(fc_w @ gap + fc_b)             # [E]
    comb    = sum_e r[e] * kernel_weights[e]         # [O, I, 3, 3]
    y[b]    = conv2d(x[b], comb, pad=1)              # [O, H, W]

Sharding: data-parallel over batch, 4 samples per core on 8 cores.
Expert kernels + fc params replicated to every core.

Direct bf16 conv (measured: PE issues one 448-free matmul per ~199ns,
so the 2*7*18 = 252 matmuls/sample are the machine floor; Winograd
trades below-floor PE work for DVE work it cannot afford). The version
differences vs the naive pipeline are all about keeping PE fed from
t=~20us on:

  - x arrives via gpsimd SWDGE cast-DMA (fp32 HBM -> bf16 SBUF,
    contiguous, no padding). Conv edge taps are handled by accumulating
    shifted partial PSUM ranges instead of padding, so there is no
    fp32 staging tile, no ACT cast pass, and no memsets.
  - W (9.4MB) is pushed as 32 (oh, ci, e) chunks alternating across
    BOTH HWDGE rings (sync+scalar), (oh0,ci0) chunks first: the first
    synthesis needs only 2.4MB landed rather than the full 9.4MB.
  - each conv oh-pass opens all 7 PSUM row-groups and runs a ci0 tap
    pass (63 matmuls) then a ci1 pass; sample 0's first synthesis is
    ci-split so conv starts after half a synthesis (~8us) instead of a
    full one.
  - GAP rides a DVE tensor_scalar copy via accum_out (~1us/half);
    routing is PE (fc matmul) -> ACT sigmoid -> PE (eye broadcast).
  - synthesis (DVE tensor_scalar 4x + tensor_tensor 2x over the bf16
    expert stack) is emitted ci-half at a time into 4 filler slots per
    sample, sized so every half lands >=8us before its conv pass.
  - PSUM->SBUF output copies on ACT, output DMA alternates rings.
"""

import numpy as np
import ml_dtypes

B, C, H, W = 32, 256, 56, 56
E = 8
N_CORES = 8
BL = B // N_CORES          # local batch per core
HWU = H * W                # 3136
ROWS = 8                   # output rows per PSUM group
NG = H // ROWS             # 7 groups per oh pass
NF = ROWS * W              # 448 matmul free dim
OIN = 128                  # output channels per half
EBLK = 2 * 2 * 9 * OIN     # per-partition free elems per expert = 4608
OHBLK = EBLK // 2          # per (oh) block = 2304
CIBLK = OHBLK // 2         # per (oh, ci) block = 1152
WP = W + 2                 # col-padded width = 58

_CACHE = {}


def _build():
    import concourse.bacc as bacc
    import concourse.mybir as mybir
    import concourse.tile as tile
    from contextlib import ExitStack

    dt = mybir.dt
    AF = mybir.ActivationFunctionType
    Alu = mybir.AluOpType

    nc = bacc.Bacc(
        "TRN2",
        target_bir_lowering=False,
        debug=False,
        enable_asserts=False,
        num_devices=N_CORES,
    )
    x_d = nc.dram_tensor("x", [BL, C, H, W], dt.float32, kind="ExternalInput")
    w_d = nc.dram_tensor("wp", [128, E * EBLK], dt.bfloat16, kind="ExternalInput")
    fcw_d = nc.dram_tensor("fcw", [C, E], dt.float32, kind="ExternalInput")
    fcb_d = nc.dram_tensor("fcb", [E, 1], dt.float32, kind="ExternalInput")
    eye_d = nc.dram_tensor("eye", [E, E], dt.float32, kind="ExternalInput")
    y_d = nc.dram_tensor("y", [BL, C, H, W], dt.float32, kind="ExternalOutput")

    with tile.TileContext(nc) as tc:
        with ExitStack() as ctx:
            cpool = ctx.enter_context(tc.tile_pool(name="consts", bufs=1))
            xvpool = ctx.enter_context(tc.tile_pool(name="xvs", bufs=3))
            xppool = ctx.enter_context(tc.tile_pool(name="xps", bufs=2))
            cbpool = ctx.enter_context(tc.tile_pool(name="cbs", bufs=2))
            opool = ctx.enter_context(tc.tile_pool(name="outs", bufs=3))
            spool = ctx.enter_context(tc.tile_pool(name="small", bufs=2))
            pspool = ctx.enter_context(tc.tile_pool(name="cpsum", bufs=7, space="PSUM"))
            psmall = ctx.enter_context(tc.tile_pool(name="spsum", bufs=1, space="PSUM"))

            w_sb = cpool.tile([128, E * EBLK], dt.bfloat16)
            fcw_sb = cpool.tile([128, 2 * E], dt.float32)
            fcb_sb = cpool.tile([E, 1], dt.float32)
            eye_sb = cpool.tile([E, E], dt.float32)

            xvs, xps, gaps, rbs, cbs = {}, {}, {}, {}, {}

            def warm_sigmoid():
                dum = spool.tile([1, 2], dt.float32, tag="dum")
                nc.vector.memset(dum[:, 0:1], 0.0)
                nc.scalar.activation(
                    out=dum[:, 1:2], in_=dum[:, 0:1], func=AF.Sigmoid,
                    bias=0.0, scale=1.0,
                )

            def load_consts():
                for ci in range(2):
                    nc.sync.dma_start(
                        out=fcw_sb[:, ci * E : (ci + 1) * E],
                        in_=fcw_d.ap()[ci * 128 : (ci + 1) * 128, :],
                    )
                nc.scalar.dma_start(out=fcb_sb[:], in_=fcb_d.ap())
                nc.scalar.dma_start(out=eye_sb[:], in_=eye_d.ap())

            def load_w():
                # 8 strided (oh, ci, e-half) chunks of 1.18MB, oh0-ci0 first.
                # Few large pushes per ring: each ring has ~8 DMA semaphore
                # slots, and a 9th+ push head-of-line blocks that engine's
                # FIFO (which stalled the routing sigmoid for 25us when W
                # went as 32 small chunks).
                rings = [nc.sync, nc.scalar, nc.sync, nc.scalar,
                         nc.gpsimd, nc.sync, nc.scalar, nc.gpsimd]
                wsv = w_sb[:].rearrange("p (e f) -> p e f", e=E)
                wdv = w_d.ap().rearrange("p (e f) -> p e f", e=E)
                k = 0
                for oh in range(2):
                    for ci in range(2):
                        lo = oh * OHBLK + ci * CIBLK
                        for e0 in (0, E // 2):
                            rings[k].dma_start(
                                out=wsv[:, e0 : e0 + E // 2, lo : lo + CIBLK],
                                in_=wdv[:, e0 : e0 + E // 2, lo : lo + CIBLK],
                            )
                            k += 1

            def stage(s):
                if s >= BL:
                    return
                xv = xvpool.tile([128, 2 * HWU], dt.bfloat16, tag="xv", name=f"xv{s}")
                xvs[s] = xv
                for ci in range(2):
                    nc.gpsimd.dma_start(
                        out=xv[:, ci * HWU : (ci + 1) * HWU],
                        in_=x_d.ap()[s, ci * 128 : (ci + 1) * 128, :, :],
                    )

            def gap_route(s):
                # col-padded bf16 copy of x (58-wide rows, zero cols 0/57) so
                # every kw tap is a full-width contiguous-dst matmul; the GAP
                # sum rides the same DVE tensor_scalar via accum_out.
                xv = xvs[s]
                xp = xppool.tile(
                    [128, 2 * H * WP], dt.bfloat16, tag="xp", name=f"xp{s}"
                )
                xps[s] = xp
                xpv = xp.rearrange("p (c h w) -> p c h w", c=2, h=H, w=WP)
                g = spool.tile([128, 2], dt.float32, tag="gap")
                gaps[s] = g
                for ci in range(2):
                    nc.vector.memset(xpv[:, ci, :, 0:1], 0.0)
                    nc.vector.memset(xpv[:, ci, :, WP - 1 : WP], 0.0)
                    nc.vector.tensor_scalar(
                        out=xpv[:, ci, :, 1 : 1 + W],
                        in0=xv[:, ci * HWU : (ci + 1) * HWU],
                        scalar1=1.0,
                        scalar2=0.0,
                        op0=Alu.mult,
                        op1=Alu.add,
                        accum_out=g[:, ci : ci + 1],
                    )
                prt = psmall.tile([128, E], dt.float32, tag="prt", name=f"prt{s}")
                for ci in range(2):
                    nc.tensor.matmul(
                        prt[0:E, 0:1],
                        lhsT=fcw_sb[:, ci * E : (ci + 1) * E],
                        rhs=g[:, ci : ci + 1],
                        start=(ci == 0),
                        stop=(ci == 1),
                    )
                rr = spool.tile([E, 1], dt.float32, tag="rr")
                nc.scalar.activation(
                    out=rr[:], in_=prt[0:E, 0:1], func=AF.Sigmoid, bias=fcb_sb[:],
                    scale=1.0,
                )
                nc.tensor.matmul(
                    prt[:],
                    lhsT=rr[:].broadcast_to([E, 128]),
                    rhs=eye_sb[:],
                    start=True,
                    stop=True,
                )
                rb = spool.tile([128, E], dt.float32, tag="rb")
                nc.scalar.activation(out=rb[:], in_=prt[:], func=AF.Copy)
                rbs[s] = rb

            def synth(s, oh, ci):
                # combined[oh,ci] = sum_e r_e * W_e[oh,ci]  (bf16, DVE)
                if s not in cbs:
                    cbs[s] = cbpool.tile([128, EBLK], dt.bfloat16, tag="cb",
                                         name=f"cb{s}")
                cb = cbs[s]
                rb = rbs[s]
                lo = oh * OHBLK + ci * CIBLK
                dstc = cb[:, lo : lo + CIBLK]
                for e in range(E):
                    src = w_sb[:, e * EBLK + lo : e * EBLK + lo + CIBLK]
                    if e == 0:
                        nc.vector.tensor_scalar_mul(dstc, src, rb[:, 0:1])
                    else:
                        tmp = spool.tile([128, CIBLK], dt.bfloat16, tag="stmp")
                        nc.vector.tensor_scalar_mul(tmp[:], src, rb[:, e : e + 1])
                        nc.vector.tensor_tensor(
                            out=dstc, in0=tmp[:], in1=dstc, op=Alu.add
                        )

            def conv_oh(s, oh, fillers):
                # 7 PSUM groups opened by a ci0 tap pass, closed by ci1.
                # Edge taps (kh at first/last group, kw 0/2 everywhere) write
                # shifted partial ranges; (kh=1,kw=1) is full-range and
                # carries the start/stop flags.
                cb = cbs[s]
                xpv = xps[s].rearrange("p (c h w) -> p c h w", c=2, h=H, w=WP)
                pss = [
                    pspool.tile([128, NF], dt.float32, tag="ps", name=f"ps{s}_{oh}_{g}")
                    for g in range(NG)
                ]

                def taps(ci, order):
                    for g in range(NG):
                        ps = pss[g]
                        psv = ps.rearrange("p (r w) -> p r w", r=ROWS, w=W)
                        r0 = g * ROWS
                        for kh, kw in order:
                            lo = oh * OHBLK + ci * CIBLK + (kh * 3 + kw) * OIN
                            rl = 1 if (g == 0 and kh == 0) else 0
                            rh = ROWS - 1 if (g == NG - 1 and kh == 2) else ROWS
                            full = rl == 0 and rh == ROWS
                            nc.tensor.matmul(
                                ps[:] if full else psv[:, rl:rh, :],
                                lhsT=cb[:, lo : lo + OIN],
                                rhs=xpv[
                                    :,
                                    ci,
                                    r0 + rl + kh - 1 : r0 + rh + kh - 1,
                                    kw : kw + W,
                                ],
                                start=(full and ci == 0 and kh == 1 and kw == 1),
                                stop=(full and ci == 1 and kh == 1 and kw == 1),
                                skip_group_check=not full,
                            )

                # ci0 pass: (1,1) first opens each group
                taps(0, [(1, 1), (0, 0), (0, 1), (0, 2), (1, 0), (1, 2),
                         (2, 0), (2, 1), (2, 2)])
                fillers[0]()
                # ci1 pass: (1,1) last closes each group
                taps(1, [(0, 0), (0, 1), (0, 2), (1, 0), (1, 2),
                         (2, 0), (2, 1), (2, 2), (1, 1)])
                for g in range(NG):
                    ot = opool.tile([128, NF], dt.float32, tag="ot")
                    nc.scalar.activation(out=ot[:], in_=pss[g][:], func=AF.Copy)
                    eng = nc.sync if g % 2 == 0 else nc.scalar
                    r0 = g * ROWS
                    eng.dma_start(
                        out=y_d.ap()[s, oh * 128 : (oh + 1) * 128, r0 : r0 + ROWS, :],
                        in_=ot[:].rearrange("p (r w) -> p r w", r=ROWS, w=W),
                    )
                fillers[1]()

            def nothing():
                pass

            # ---- software-pipelined emission ----
            warm_sigmoid()
            load_consts()
            stage(0)
            load_w()
            stage(1)
            gap_route(0)
            synth(0, 0, 0)
            synth(0, 0, 1)
            for s in range(BL):
                nxt = s + 1
                have_next = nxt < BL
                conv_oh(s, 0, [
                    lambda s=s: synth(s, 1, 0),
                    lambda s=s: synth(s, 1, 1),
                ])
                conv_oh(s, 1, [
                    (lambda n=nxt: (gap_route(n), synth(n, 0, 0)))
                    if have_next else nothing,
                    (lambda n=nxt: (synth(n, 0, 1), stage(n + 1)))
                    if have_next else nothing,
                ])

    nc.compile()
    return nc


def _get_nc():
    if "nc" not in _CACHE:
        _CACHE["nc"] = _build()
    return _CACHE["nc"]


def _pack_inputs(x, kernel_weights, fc_w, fc_b):
    # w layout per partition p (= i % 128): [e, oh, ci, kh, kw, oin], bf16
    a = np.asarray(kernel_weights, np.float32).reshape(E, 2, 128, 2, 128, 3, 3)
    # dims: e, oh, oin, ci, p, kh, kw -> p, e, oh, ci, kh, kw, oin
    a = np.ascontiguousarray(a.transpose(4, 0, 1, 3, 5, 6, 2)).reshape(128, E * EBLK)
    wp = a.astype(ml_dtypes.bfloat16)
    fcw_t = np.ascontiguousarray(np.asarray(fc_w, np.float32).T / float(H * W))
    fcb2 = np.ascontiguousarray(np.asarray(fc_b, np.float32).reshape(E, 1))
    eye = np.eye(E, dtype=np.float32)
    x = np.ascontiguousarray(np.asarray(x, np.float32))
    in_maps = []
    for i in range(N_CORES):
        in_maps.append(
            {
                "x": x[i * BL : (i + 1) * BL],
                "wp": wp,
                "fcw": fcw_t,
                "fcb": fcb2,
                "eye": eye,
            }
        )
    return in_maps


def _run(x, kernel_weights, fc_w, fc_b, trace=False):
    from concourse.bass_utils import run_bass_kernel_spmd

    nc = _get_nc()
    in_maps = _pack_inputs(x, kernel_weights, fc_w, fc_b)
    res = run_bass_kernel_spmd(nc, in_maps, core_ids=list(range(N_CORES)), trace=trace)
    y = np.concatenate([res.results[i]["y"] for i in range(N_CORES)], axis=0)
    return np.ascontiguousarray(y.astype(np.float32)), res


def kernel(x, kernel_weights, fc_w, fc_b):
    y, _ = _run(x, kernel_weights, fc_w, fc_b, trace=False)
    return y


def kernel_traced(x, kernel_weights, fc_w, fc_b):
    y, res = _run(x, kernel_weights, fc_w, fc_b, trace=True)
    return y, res


# revision 14
# speedup vs baseline: 1.1350x; 1.1350x over previous
"""CondConv (per-sample dynamic conv) Trainium2 Bass kernel.

Reference computation (per sample b):
    gap     = mean(x[b], spatial)                    # [C]
    r       = sigmoid(fc_w @ gap + fc_b)             # [E]
    comb    = sum_e r[e] * kernel_weights[e]         # [O, I, 3, 3]
    y[b]    = conv2d(x[b], comb, pad=1)              # [O, H, W]

Sharding: data-parallel over batch, 4 samples per core on 8 cores.
Expert kernels + fc params replicated to every core.

Direct bf16 conv (measured: PE issues one 448-free matmul per ~199ns,
so the 2*7*18 = 252 matmuls/sample are the machine floor; Winograd
trades below-floor PE work for DVE work it cannot afford). The version
differences vs the naive pipeline are all about keeping PE fed from
t=~20us on:

  - x arrives via gpsimd SWDGE cast-DMA (fp32 HBM -> bf16 SBUF,
    contiguous, no padding). Conv edge taps are handled by accumulating
    shifted partial PSUM ranges instead of padding, so there is no
    fp32 staging tile, no ACT cast pass, and no memsets.
  - W (9.4MB) is pushed as 32 (oh, ci, e) chunks alternating across
    BOTH HWDGE rings (sync+scalar), (oh0,ci0) chunks first: the first
    synthesis needs only 2.4MB landed rather than the full 9.4MB.
  - each conv oh-pass opens all 7 PSUM row-groups and runs a ci0 tap
    pass (63 matmuls) then a ci1 pass; sample 0's first synthesis is
    ci-split so conv starts after half a synthesis (~8us) instead of a
    full one.
  - GAP rides a DVE tensor_scalar copy via accum_out (~1us/half);
    routing is PE (fc matmul) -> ACT sigmoid -> PE (eye broadcast).
  - synthesis (DVE tensor_scalar 4x + tensor_tensor 2x over the bf16
    expert stack) is emitted ci-half at a time into 4 filler slots per
    sample, sized so every half lands >=8us before its conv pass.
  - PSUM->SBUF output copies on ACT, output DMA alternates rings.
"""

import numpy as np
import ml_dtypes

B, C, H, W = 32, 256, 56, 56
E = 8
N_CORES = 8
BL = B // N_CORES          # local batch per core
HWU = H * W                # 3136
ROWS = 8                   # output rows per PSUM group
NG = H // ROWS             # 7 groups per oh pass
NF = ROWS * W              # 448 matmul free dim
OIN = 128                  # output channels per half
EBLK = 2 * 2 * 9 * OIN     # per-partition free elems per expert = 4608
OHBLK = EBLK // 2          # per (oh) block = 2304
CIBLK = OHBLK // 2         # per (oh, ci) block = 1152
WP = W + 2                 # col-padded width = 58

_CACHE = {}


def _build():
    import concourse.bacc as bacc
    import concourse.mybir as mybir
    import concourse.tile as tile
    from contextlib import ExitStack

    dt = mybir.dt
    AF = mybir.ActivationFunctionType
    Alu = mybir.AluOpType

    nc = bacc.Bacc(
        "TRN2",
        target_bir_lowering=False,
        debug=False,
        enable_asserts=False,
        num_devices=N_CORES,
    )
    x_d = nc.dram_tensor("x", [BL, C, H, W], dt.float32, kind="ExternalInput")
    w_d = nc.dram_tensor("wp", [128, E * EBLK], dt.bfloat16, kind="ExternalInput")
    fcw_d = nc.dram_tensor("fcw", [C, E], dt.float32, kind="ExternalInput")
    fcb_d = nc.dram_tensor("fcb", [E, 1], dt.float32, kind="ExternalInput")
    eye_d = nc.dram_tensor("eye", [E, E], dt.float32, kind="ExternalInput")
    y_d = nc.dram_tensor("y", [BL, C, H, W], dt.float32, kind="ExternalOutput")

    with tile.TileContext(nc) as tc:
        with ExitStack() as ctx:
            cpool = ctx.enter_context(tc.tile_pool(name="consts", bufs=1))
            xvpool = ctx.enter_context(tc.tile_pool(name="xvs", bufs=3))
            xppool = ctx.enter_context(tc.tile_pool(name="xps", bufs=2))
            cbpool = ctx.enter_context(tc.tile_pool(name="cbs", bufs=2))
            opool = ctx.enter_context(tc.tile_pool(name="outs", bufs=3))
            spool = ctx.enter_context(tc.tile_pool(name="small", bufs=2))
            pspool = ctx.enter_context(tc.tile_pool(name="cpsum", bufs=7, space="PSUM"))
            psmall = ctx.enter_context(tc.tile_pool(name="spsum", bufs=1, space="PSUM"))

            w_sb = cpool.tile([128, E * EBLK], dt.bfloat16)
            fcw_sb = cpool.tile([128, 2 * E], dt.float32)
            fcb_sb = cpool.tile([E, 1], dt.float32)
            eye_sb = cpool.tile([E, E], dt.float32)

            xvs, xps, gaps, rbs, cbs = {}, {}, {}, {}, {}

            def warm_sigmoid():
                dum = spool.tile([1, 2], dt.float32, tag="dum")
                nc.vector.memset(dum[:, 0:1], 0.0)
                nc.scalar.activation(
                    out=dum[:, 1:2], in_=dum[:, 0:1], func=AF.Sigmoid,
                    bias=0.0, scale=1.0,
                )

            def load_consts():
                for ci in range(2):
                    nc.sync.dma_start(
                        out=fcw_sb[:, ci * E : (ci + 1) * E],
                        in_=fcw_d.ap()[ci * 128 : (ci + 1) * 128, :],
                    )
                nc.scalar.dma_start(out=fcb_sb[:], in_=fcb_d.ap())
                nc.scalar.dma_start(out=eye_sb[:], in_=eye_d.ap())

            def load_w(ohs, rings):
                # one fully-contiguous 2.36MB chunk per (oh, ci): per-
                # partition segment is 18.4KB, so the DMA runs at full rate
                # (strided 2.3KB-segment chunks measured ~3x slower).
                for i, oh in enumerate(ohs):
                    for ci in range(2):
                        lo = (oh * 2 + ci) * E * CIBLK
                        ring = rings[i * 2 + ci]
                        ring.dma_start(
                            out=w_sb[:, lo : lo + E * CIBLK],
                            in_=w_d.ap()[:, lo : lo + E * CIBLK],
                        )

            def stage(s):
                if s >= BL:
                    return
                xv = xvpool.tile([128, 2 * HWU], dt.bfloat16, tag="xv", name=f"xv{s}")
                xvs[s] = xv
                for ci in range(2):
                    nc.gpsimd.dma_start(
                        out=xv[:, ci * HWU : (ci + 1) * HWU],
                        in_=x_d.ap()[s, ci * 128 : (ci + 1) * 128, :, :],
                    )

            def gap_route(s):
                # col-padded bf16 copy of x (58-wide rows, zero cols 0/57) so
                # every kw tap is a full-width contiguous-dst matmul; the GAP
                # sum rides the same DVE tensor_scalar via accum_out.
                xv = xvs[s]
                xp = xppool.tile(
                    [128, 2 * H * WP], dt.bfloat16, tag="xp", name=f"xp{s}"
                )
                xps[s] = xp
                xpv = xp.rearrange("p (c h w) -> p c h w", c=2, h=H, w=WP)
                g = spool.tile([128, 2], dt.float32, tag="gap")
                gaps[s] = g
                for ci in range(2):
                    nc.vector.memset(xpv[:, ci, :, 0:1], 0.0)
                    nc.vector.memset(xpv[:, ci, :, WP - 1 : WP], 0.0)
                    nc.vector.tensor_scalar(
                        out=xpv[:, ci, :, 1 : 1 + W],
                        in0=xv[:, ci * HWU : (ci + 1) * HWU],
                        scalar1=1.0,
                        scalar2=0.0,
                        op0=Alu.mult,
                        op1=Alu.add,
                        accum_out=g[:, ci : ci + 1],
                    )
                prt = psmall.tile([128, E], dt.float32, tag="prt", name=f"prt{s}")
                for ci in range(2):
                    nc.tensor.matmul(
                        prt[0:E, 0:1],
                        lhsT=fcw_sb[:, ci * E : (ci + 1) * E],
                        rhs=g[:, ci : ci + 1],
                        start=(ci == 0),
                        stop=(ci == 1),
                    )
                rr = spool.tile([E, 1], dt.float32, tag="rr")
                nc.scalar.activation(
                    out=rr[:], in_=prt[0:E, 0:1], func=AF.Sigmoid, bias=fcb_sb[:],
                    scale=1.0,
                )
                nc.tensor.matmul(
                    prt[:],
                    lhsT=rr[:].broadcast_to([E, 128]),
                    rhs=eye_sb[:],
                    start=True,
                    stop=True,
                )
                rb = spool.tile([128, E], dt.float32, tag="rb")
                nc.scalar.activation(out=rb[:], in_=prt[:], func=AF.Copy)
                rbs[s] = rb

            def synth(s, oh, ci):
                # combined[oh,ci] = sum_e r_e * W_e[oh,ci]  (bf16, DVE)
                if s not in cbs:
                    cbs[s] = cbpool.tile([128, EBLK], dt.bfloat16, tag="cb",
                                         name=f"cb{s}")
                cb = cbs[s]
                rb = rbs[s]
                lo = oh * OHBLK + ci * CIBLK
                dstc = cb[:, lo : lo + CIBLK]
                for e in range(E):
                    so = ((oh * 2 + ci) * E + e) * CIBLK
                    src = w_sb[:, so : so + CIBLK]
                    if e == 0:
                        nc.vector.tensor_scalar_mul(dstc, src, rb[:, 0:1])
                    else:
                        tmp = spool.tile([128, CIBLK], dt.bfloat16, tag="stmp")
                        nc.vector.tensor_scalar_mul(tmp[:], src, rb[:, e : e + 1])
                        nc.vector.tensor_tensor(
                            out=dstc, in0=tmp[:], in1=dstc, op=Alu.add
                        )

            def conv_oh(s, oh, fillers):
                # 7 PSUM groups opened by a ci0 tap pass, closed by ci1.
                # Edge taps (kh at first/last group, kw 0/2 everywhere) write
                # shifted partial ranges; (kh=1,kw=1) is full-range and
                # carries the start/stop flags.
                cb = cbs[s]
                xpv = xps[s].rearrange("p (c h w) -> p c h w", c=2, h=H, w=WP)
                pss = [
                    pspool.tile([128, NF], dt.float32, tag="ps", name=f"ps{s}_{oh}_{g}")
                    for g in range(NG)
                ]

                def taps(ci, order):
                    for g in range(NG):
                        ps = pss[g]
                        psv = ps.rearrange("p (r w) -> p r w", r=ROWS, w=W)
                        r0 = g * ROWS
                        for kh, kw in order:
                            lo = oh * OHBLK + ci * CIBLK + (kh * 3 + kw) * OIN
                            rl = 1 if (g == 0 and kh == 0) else 0
                            rh = ROWS - 1 if (g == NG - 1 and kh == 2) else ROWS
                            full = rl == 0 and rh == ROWS
                            nc.tensor.matmul(
                                ps[:] if full else psv[:, rl:rh, :],
                                lhsT=cb[:, lo : lo + OIN],
                                rhs=xpv[
                                    :,
                                    ci,
                                    r0 + rl + kh - 1 : r0 + rh + kh - 1,
                                    kw : kw + W,
                                ],
                                start=(full and ci == 0 and kh == 1 and kw == 1),
                                stop=(full and ci == 1 and kh == 1 and kw == 1),
                                skip_group_check=not full,
                            )

                # ci0 pass: (1,1) first opens each group
                taps(0, [(1, 1), (0, 0), (0, 1), (0, 2), (1, 0), (1, 2),
                         (2, 0), (2, 1), (2, 2)])
                fillers[0]()
                # ci1 pass: (1,1) last closes each group
                taps(1, [(0, 0), (0, 1), (0, 2), (1, 0), (1, 2),
                         (2, 0), (2, 1), (2, 2), (1, 1)])
                for g in range(NG):
                    ot = opool.tile([128, NF], dt.float32, tag="ot")
                    nc.scalar.activation(out=ot[:], in_=pss[g][:], func=AF.Copy)
                    eng = nc.sync if g % 2 == 0 else nc.scalar
                    r0 = g * ROWS
                    eng.dma_start(
                        out=y_d.ap()[s, oh * 128 : (oh + 1) * 128, r0 : r0 + ROWS, :],
                        in_=ot[:].rearrange("p (r w) -> p r w", r=ROWS, w=W),
                    )
                fillers[1]()

            def nothing():
                pass

            # ---- software-pipelined emission ----
            warm_sigmoid()
            load_consts()
            stage(0)
            load_w([0], [nc.sync, nc.scalar])
            load_w([1], [nc.gpsimd, nc.gpsimd])
            stage(1)
            gap_route(0)
            synth(0, 0, 0)
            synth(0, 0, 1)
            for s in range(BL):
                nxt = s + 1
                have_next = nxt < BL
                conv_oh(s, 0, [
                    lambda s=s: synth(s, 1, 0),
                    lambda s=s: synth(s, 1, 1),
                ])
                conv_oh(s, 1, [
                    (lambda n=nxt: (gap_route(n), synth(n, 0, 0)))
                    if have_next else nothing,
                    (lambda n=nxt: (synth(n, 0, 1), stage(n + 1)))
                    if have_next else nothing,
                ])

    nc.compile()
    return nc


def _get_nc():
    if "nc" not in _CACHE:
        _CACHE["nc"] = _build()
    return _CACHE["nc"]


def _pack_inputs(x, kernel_weights, fc_w, fc_b):
    # w layout per partition p (= i % 128): [oh, ci, e, kh, kw, oin], bf16
    # -- (oh, ci)-major so one W DMA chunk is a fully contiguous
    # 18.4KB-per-partition run (DMA rate scales with segment size)
    a = np.asarray(kernel_weights, np.float32).reshape(E, 2, 128, 2, 128, 3, 3)
    # dims: e, oh, oin, ci, p, kh, kw -> p, oh, ci, e, kh, kw, oin
    a = np.ascontiguousarray(a.transpose(4, 1, 3, 0, 5, 6, 2)).reshape(128, E * EBLK)
    wp = a.astype(ml_dtypes.bfloat16)
    fcw_t = np.ascontiguousarray(np.asarray(fc_w, np.float32).T / float(H * W))
    fcb2 = np.ascontiguousarray(np.asarray(fc_b, np.float32).reshape(E, 1))
    eye = np.eye(E, dtype=np.float32)
    x = np.ascontiguousarray(np.asarray(x, np.float32))
    in_maps = []
    for i in range(N_CORES):
        in_maps.append(
            {
                "x": x[i * BL : (i + 1) * BL],
                "wp": wp,
                "fcw": fcw_t,
                "fcb": fcb2,
                "eye": eye,
            }
        )
    return in_maps


def _run(x, kernel_weights, fc_w, fc_b, trace=False):
    from concourse.bass_utils import run_bass_kernel_spmd

    nc = _get_nc()
    in_maps = _pack_inputs(x, kernel_weights, fc_w, fc_b)
    res = run_bass_kernel_spmd(nc, in_maps, core_ids=list(range(N_CORES)), trace=trace)
    y = np.concatenate([res.results[i]["y"] for i in range(N_CORES)], axis=0)
    return np.ascontiguousarray(y.astype(np.float32)), res


def kernel(x, kernel_weights, fc_w, fc_b):
    y, _ = _run(x, kernel_weights, fc_w, fc_b, trace=False)
    return y


def kernel_traced(x, kernel_weights, fc_w, fc_b):
    y, res = _run(x, kernel_weights, fc_w, fc_b, trace=True)
    return y, res


# revision 15
# speedup vs baseline: 1.1678x; 1.0289x over previous
"""CondConv (per-sample dynamic conv) Trainium2 Bass kernel.

Reference computation (per sample b):
    gap     = mean(x[b], spatial)                    # [C]
    r       = sigmoid(fc_w @ gap + fc_b)             # [E]
    comb    = sum_e r[e] * kernel_weights[e]         # [O, I, 3, 3]
    y[b]    = conv2d(x[b], comb, pad=1)              # [O, H, W]

Sharding: data-parallel over batch, 4 samples per core on 8 cores.
Expert kernels + fc params replicated to every core.

Direct bf16 conv (measured: PE issues one 448-free matmul per ~199ns,
so the 2*7*18 = 252 matmuls/sample are the machine floor; Winograd
trades below-floor PE work for DVE work it cannot afford). The version
differences vs the naive pipeline are all about keeping PE fed from
t=~20us on:

  - x arrives via gpsimd SWDGE cast-DMA (fp32 HBM -> bf16 SBUF,
    contiguous, no padding). Conv edge taps are handled by accumulating
    shifted partial PSUM ranges instead of padding, so there is no
    fp32 staging tile, no ACT cast pass, and no memsets.
  - W (9.4MB) is pushed as 32 (oh, ci, e) chunks alternating across
    BOTH HWDGE rings (sync+scalar), (oh0,ci0) chunks first: the first
    synthesis needs only 2.4MB landed rather than the full 9.4MB.
  - each conv oh-pass opens all 7 PSUM row-groups and runs a ci0 tap
    pass (63 matmuls) then a ci1 pass; sample 0's first synthesis is
    ci-split so conv starts after half a synthesis (~8us) instead of a
    full one.
  - GAP rides a DVE tensor_scalar copy via accum_out (~1us/half);
    routing is PE (fc matmul) -> ACT sigmoid -> PE (eye broadcast).
  - synthesis (DVE tensor_scalar 4x + tensor_tensor 2x over the bf16
    expert stack) is emitted ci-half at a time into 4 filler slots per
    sample, sized so every half lands >=8us before its conv pass.
  - PSUM->SBUF output copies on ACT, output DMA alternates rings.
"""

import numpy as np
import ml_dtypes

B, C, H, W = 32, 256, 56, 56
E = 8
N_CORES = 8
BL = B // N_CORES          # local batch per core
HWU = H * W                # 3136
ROWS = 8                   # output rows per PSUM group
NG = H // ROWS             # 7 groups per oh pass
NF = ROWS * W              # 448 matmul free dim
OIN = 128                  # output channels per half
EBLK = 2 * 2 * 9 * OIN     # per-partition free elems per expert = 4608
OHBLK = EBLK // 2          # per (oh) block = 2304
CIBLK = OHBLK // 2         # per (oh, ci) block = 1152
WP = W + 2                 # col-padded width = 58

_CACHE = {}


def _build():
    import concourse.bacc as bacc
    import concourse.mybir as mybir
    import concourse.tile as tile
    from contextlib import ExitStack

    dt = mybir.dt
    AF = mybir.ActivationFunctionType
    Alu = mybir.AluOpType

    nc = bacc.Bacc(
        "TRN2",
        target_bir_lowering=False,
        debug=False,
        enable_asserts=False,
        num_devices=N_CORES,
    )
    x_d = nc.dram_tensor("x", [BL, C, H, W], dt.float32, kind="ExternalInput")
    w_d = nc.dram_tensor("wp", [128, E * EBLK], dt.bfloat16, kind="ExternalInput")
    fcw_d = nc.dram_tensor("fcw", [C, E], dt.float32, kind="ExternalInput")
    fcb_d = nc.dram_tensor("fcb", [E, 1], dt.float32, kind="ExternalInput")
    eye_d = nc.dram_tensor("eye", [E, E], dt.float32, kind="ExternalInput")
    y_d = nc.dram_tensor("y", [BL, C, H, W], dt.float32, kind="ExternalOutput")

    with tile.TileContext(nc) as tc:
        with ExitStack() as ctx:
            cpool = ctx.enter_context(tc.tile_pool(name="consts", bufs=1))
            xvpool = ctx.enter_context(tc.tile_pool(name="xvs", bufs=3))
            xppool = ctx.enter_context(tc.tile_pool(name="xps", bufs=2))
            cbpool = ctx.enter_context(tc.tile_pool(name="cbs", bufs=2))
            opool = ctx.enter_context(tc.tile_pool(name="outs", bufs=3))
            spool = ctx.enter_context(tc.tile_pool(name="small", bufs=2))
            pspool = ctx.enter_context(tc.tile_pool(name="cpsum", bufs=7, space="PSUM"))
            psmall = ctx.enter_context(tc.tile_pool(name="spsum", bufs=1, space="PSUM"))

            w_sb = cpool.tile([128, E * EBLK], dt.bfloat16)
            fcw_sb = cpool.tile([128, 2 * E], dt.float32)
            fcb_sb = cpool.tile([E, 1], dt.float32)
            eye_sb = cpool.tile([E, E], dt.float32)

            xvs, xps, gaps, rbs, cbs = {}, {}, {}, {}, {}

            def warm_sigmoid():
                dum = spool.tile([1, 2], dt.float32, tag="dum")
                nc.vector.memset(dum[:, 0:1], 0.0)
                nc.scalar.activation(
                    out=dum[:, 1:2], in_=dum[:, 0:1], func=AF.Sigmoid,
                    bias=0.0, scale=1.0,
                )

            def load_consts():
                for ci in range(2):
                    nc.sync.dma_start(
                        out=fcw_sb[:, ci * E : (ci + 1) * E],
                        in_=fcw_d.ap()[ci * 128 : (ci + 1) * 128, :],
                    )
                nc.scalar.dma_start(out=fcb_sb[:], in_=fcb_d.ap())
                nc.scalar.dma_start(out=eye_sb[:], in_=eye_d.ap())

            def load_w_chunk(oh, ci, ring, half=None):
                # fully-contiguous chunks: per-partition segment is 9.2 or
                # 18.4KB, so the DMA runs at full rate (strided 2.3KB-segment
                # chunks measured ~3x slower).
                lo = (oh * 2 + ci) * E * CIBLK
                n = E * CIBLK
                if half is not None:
                    n //= 2
                    lo += half * n
                ring.dma_start(
                    out=w_sb[:, lo : lo + n],
                    in_=w_d.ap()[:, lo : lo + n],
                )

            def stage(s):
                if s >= BL:
                    return
                xv = xvpool.tile([128, 2 * HWU], dt.bfloat16, tag="xv", name=f"xv{s}")
                xvs[s] = xv
                for ci in range(2):
                    nc.gpsimd.dma_start(
                        out=xv[:, ci * HWU : (ci + 1) * HWU],
                        in_=x_d.ap()[s, ci * 128 : (ci + 1) * 128, :, :],
                    )

            def gap_route(s):
                # col-padded bf16 copy of x (58-wide rows, zero cols 0/57) so
                # every kw tap is a full-width contiguous-dst matmul; the GAP
                # sum rides the same DVE tensor_scalar via accum_out.
                xv = xvs[s]
                xp = xppool.tile(
                    [128, 2 * H * WP], dt.bfloat16, tag="xp", name=f"xp{s}"
                )
                xps[s] = xp
                xpv = xp.rearrange("p (c h w) -> p c h w", c=2, h=H, w=WP)
                g = spool.tile([128, 2], dt.float32, tag="gap")
                gaps[s] = g
                for ci in range(2):
                    nc.vector.memset(xpv[:, ci, :, 0:1], 0.0)
                    nc.vector.memset(xpv[:, ci, :, WP - 1 : WP], 0.0)
                    nc.vector.tensor_scalar(
                        out=xpv[:, ci, :, 1 : 1 + W],
                        in0=xv[:, ci * HWU : (ci + 1) * HWU],
                        scalar1=1.0,
                        scalar2=0.0,
                        op0=Alu.mult,
                        op1=Alu.add,
                        accum_out=g[:, ci : ci + 1],
                    )
                prt = psmall.tile([128, E], dt.float32, tag="prt", name=f"prt{s}")
                for ci in range(2):
                    nc.tensor.matmul(
                        prt[0:E, 0:1],
                        lhsT=fcw_sb[:, ci * E : (ci + 1) * E],
                        rhs=g[:, ci : ci + 1],
                        start=(ci == 0),
                        stop=(ci == 1),
                    )
                rr = spool.tile([E, 1], dt.float32, tag="rr")
                nc.scalar.activation(
                    out=rr[:], in_=prt[0:E, 0:1], func=AF.Sigmoid, bias=fcb_sb[:],
                    scale=1.0,
                )
                nc.tensor.matmul(
                    prt[:],
                    lhsT=rr[:].broadcast_to([E, 128]),
                    rhs=eye_sb[:],
                    start=True,
                    stop=True,
                )
                rb = spool.tile([128, E], dt.float32, tag="rb")
                nc.scalar.activation(out=rb[:], in_=prt[:], func=AF.Copy)
                rbs[s] = rb

            def synth(s, oh, ci):
                # combined[oh,ci] = sum_e r_e * W_e[oh,ci]  (bf16, DVE)
                if s not in cbs:
                    cbs[s] = cbpool.tile([128, EBLK], dt.bfloat16, tag="cb",
                                         name=f"cb{s}")
                cb = cbs[s]
                rb = rbs[s]
                lo = oh * OHBLK + ci * CIBLK
                dstc = cb[:, lo : lo + CIBLK]
                for e in range(E):
                    so = ((oh * 2 + ci) * E + e) * CIBLK
                    src = w_sb[:, so : so + CIBLK]
                    if e == 0:
                        nc.vector.tensor_scalar_mul(dstc, src, rb[:, 0:1])
                    else:
                        tmp = spool.tile([128, CIBLK], dt.bfloat16, tag="stmp")
                        nc.vector.tensor_scalar_mul(tmp[:], src, rb[:, e : e + 1])
                        nc.vector.tensor_tensor(
                            out=dstc, in0=tmp[:], in1=dstc, op=Alu.add
                        )

            def conv_oh(s, oh, fillers):
                # 7 PSUM groups opened by a ci0 tap pass, closed by ci1.
                # Edge taps (kh at first/last group, kw 0/2 everywhere) write
                # shifted partial ranges; (kh=1,kw=1) is full-range and
                # carries the start/stop flags.
                cb = cbs[s]
                xpv = xps[s].rearrange("p (c h w) -> p c h w", c=2, h=H, w=WP)
                pss = [
                    pspool.tile([128, NF], dt.float32, tag="ps", name=f"ps{s}_{oh}_{g}")
                    for g in range(NG)
                ]

                def taps(ci, order):
                    for g in range(NG):
                        ps = pss[g]
                        psv = ps.rearrange("p (r w) -> p r w", r=ROWS, w=W)
                        r0 = g * ROWS
                        for kh, kw in order:
                            lo = oh * OHBLK + ci * CIBLK + (kh * 3 + kw) * OIN
                            rl = 1 if (g == 0 and kh == 0) else 0
                            rh = ROWS - 1 if (g == NG - 1 and kh == 2) else ROWS
                            full = rl == 0 and rh == ROWS
                            nc.tensor.matmul(
                                ps[:] if full else psv[:, rl:rh, :],
                                lhsT=cb[:, lo : lo + OIN],
                                rhs=xpv[
                                    :,
                                    ci,
                                    r0 + rl + kh - 1 : r0 + rh + kh - 1,
                                    kw : kw + W,
                                ],
                                start=(full and ci == 0 and kh == 1 and kw == 1),
                                stop=(full and ci == 1 and kh == 1 and kw == 1),
                                skip_group_check=not full,
                            )

                # ci0 pass: (1,1) first opens each group
                taps(0, [(1, 1), (0, 0), (0, 1), (0, 2), (1, 0), (1, 2),
                         (2, 0), (2, 1), (2, 2)])
                fillers[0]()
                # ci1 pass: (1,1) last closes each group
                taps(1, [(0, 0), (0, 1), (0, 2), (1, 0), (1, 2),
                         (2, 0), (2, 1), (2, 2), (1, 1)])
                for g in range(NG):
                    ot = opool.tile([128, NF], dt.float32, tag="ot")
                    nc.scalar.activation(out=ot[:], in_=pss[g][:], func=AF.Copy)
                    eng = nc.sync if g % 2 == 0 else nc.scalar
                    r0 = g * ROWS
                    eng.dma_start(
                        out=y_d.ap()[s, oh * 128 : (oh + 1) * 128, r0 : r0 + ROWS, :],
                        in_=ot[:].rearrange("p (r w) -> p r w", r=ROWS, w=W),
                    )
                fillers[1]()

            def nothing():
                pass

            # ---- software-pipelined emission ----
            warm_sigmoid()
            load_consts()
            stage(0)
            # W-oh0ci0 halves on the HWDGE rings; the rest queues on gpsimd
            # BEHIND x0 so x0 gets maximum DMA-engine share early.
            load_w_chunk(0, 0, nc.sync, half=0)
            load_w_chunk(0, 0, nc.scalar, half=1)
            load_w_chunk(0, 1, nc.gpsimd)
            load_w_chunk(1, 0, nc.gpsimd)
            load_w_chunk(1, 1, nc.gpsimd)
            stage(1)
            gap_route(0)
            synth(0, 0, 0)
            synth(0, 0, 1)
            for s in range(BL):
                nxt = s + 1
                have_next = nxt < BL
                conv_oh(s, 0, [
                    lambda s=s: synth(s, 1, 0),
                    lambda s=s: synth(s, 1, 1),
                ])
                conv_oh(s, 1, [
                    (lambda n=nxt: (gap_route(n), synth(n, 0, 0)))
                    if have_next else nothing,
                    (lambda n=nxt: (synth(n, 0, 1), stage(n + 1)))
                    if have_next else nothing,
                ])

    nc.compile()
    return nc


def _get_nc():
    if "nc" not in _CACHE:
        _CACHE["nc"] = _build()
    return _CACHE["nc"]


def _pack_inputs(x, kernel_weights, fc_w, fc_b):
    # w layout per partition p (= i % 128): [oh, ci, e, kh, kw, oin], bf16
    # -- (oh, ci)-major so one W DMA chunk is a fully contiguous
    # 18.4KB-per-partition run (DMA rate scales with segment size)
    a = np.asarray(kernel_weights, np.float32).reshape(E, 2, 128, 2, 128, 3, 3)
    # dims: e, oh, oin, ci, p, kh, kw -> p, oh, ci, e, kh, kw, oin
    a = np.ascontiguousarray(a.transpose(4, 1, 3, 0, 5, 6, 2)).reshape(128, E * EBLK)
    wp = a.astype(ml_dtypes.bfloat16)
    fcw_t = np.ascontiguousarray(np.asarray(fc_w, np.float32).T / float(H * W))
    fcb2 = np.ascontiguousarray(np.asarray(fc_b, np.float32).reshape(E, 1))
    eye = np.eye(E, dtype=np.float32)
    x = np.ascontiguousarray(np.asarray(x, np.float32))
    in_maps = []
    for i in range(N_CORES):
        in_maps.append(
            {
                "x": x[i * BL : (i + 1) * BL],
                "wp": wp,
                "fcw": fcw_t,
                "fcb": fcb2,
                "eye": eye,
            }
        )
    return in_maps


def _run(x, kernel_weights, fc_w, fc_b, trace=False):
    from concourse.bass_utils import run_bass_kernel_spmd

    nc = _get_nc()
    in_maps = _pack_inputs(x, kernel_weights, fc_w, fc_b)
    res = run_bass_kernel_spmd(nc, in_maps, core_ids=list(range(N_CORES)), trace=trace)
    y = np.concatenate([res.results[i]["y"] for i in range(N_CORES)], axis=0)
    return np.ascontiguousarray(y.astype(np.float32)), res


def kernel(x, kernel_weights, fc_w, fc_b):
    y, _ = _run(x, kernel_weights, fc_w, fc_b, trace=False)
    return y


def kernel_traced(x, kernel_weights, fc_w, fc_b):
    y, res = _run(x, kernel_weights, fc_w, fc_b, trace=True)
    return y, res
